# Initial kernel scaffold
#
"""CTC total-loss kernel for Trainium2 (8 NeuronCores, Bass/Tile).

Strategy (data-parallel over batch, 4 examples per core):

 * The softmax denominator decouples from the CTC alpha recursion in the
   probability domain:  loss_b = -log(l1u + l2u) + sum_{t<al} lse[t,b] - tilt,
   where l*u come from an UNNORMALIZED recursion over exp(acts at lattice
   labels).  So each core runs two independent pipelines:
     1. stream its 33.5MB acts slab once, computing per-(t,b) sum(exp(acts))
        with a single fused ACT Exp+accum instruction per (128,4096) tile;
     2. gather the 33 lattice vocab columns per example (indirect DMA),
        and run the alpha recursion.
 * The alpha recursion is computed s-major: column s over all t is a
   first-order linear recurrence x_t = E_t * (x_{t-1} + u_t), one
   tensor_tensor_scan instruction per (column, half).  65 columns replace
   512 serial timesteps; shifts in s are free AP offsets.
 * f32 dynamic range is controlled by a per-(b,t) exponential tilt
   gamma_t = max_j(gathered acts)[t] + C_TILT, plus one renormalization of
   the boundary state at t=256 to a mid-window target.  Correction terms
   (cumsum of gamma, renorm factor) are outputs; the host folds them back
   in log domain (validated margins ~>8 nats to f32 limits on the
   reference input distribution).

The device program is input-independent (all data dependence flows through
input tensors), so it SPMDs across the 8 cores and compiles once.
Host work is index prep (labels -> gather indices / skip masks) and the
final ~100-flop log-domain assembly of the scalar loss.
"""

import numpy as np

import concourse.bass as bass
import concourse.bacc as bacc
import concourse.tile as tile
from concourse import mybir

F32 = mybir.dt.float32
BF16 = mybir.dt.bfloat16
I32 = mybir.dt.int32

T, B, V, LMAX = 512, 32, 4096, 32
NCORES = 8
BC = B // NCORES            # 4 examples per core
S = 2 * LMAX + 1            # 65 lattice states
J = LMAX + 1                # 33 gathered vocab slots (blank + labels)
TH = T // 2                 # 256: renorm halfway
NT = (T * BC) // 128        # 16 stream tiles of (128, V)
RING = 8                    # ring depth (columns kept in SBUF)
C_TILT = -1.20              # tilt constant on top of per-t max
TB_LOG = 58.0               # renorm boundary target: max -> e^TB_LOG
CHUNKS = 8                  # gather chunks (j-aligned)

_CACHE = {}


def _j_chunks():
    # split J=33 slots into CHUNKS j-aligned chunks
    base = J // CHUNKS
    sizes = [base] * CHUNKS
    for i in range(J - base * CHUNKS):
        sizes[i] += 1
    out = []
    j0 = 0
    for sz in sizes:
        out.append((j0, sz))
        j0 += sz
    return out


def _build_nc():
    nc = bacc.Bacc(None)
    acts_d = nc.dram_tensor("acts", [T, BC, V], F32, kind="ExternalInput")
    gidx_d = nc.dram_tensor("gidx", [BC, J * T], I32, kind="ExternalInput")
    skipm_d = nc.dram_tensor("skipm", [BC, S], F32, kind="ExternalInput")
    xcols_d = nc.dram_tensor("xcols", [S, BC, T], F32, kind="ExternalOutput")
    gcum_d = nc.dram_tensor("gcum", [BC, T], F32, kind="ExternalOutput")
    rfac_d = nc.dram_tensor("rfac", [BC, 1], F32, kind="ExternalOutput")
    sums_d = nc.dram_tensor("sums", [128, NT], F32, kind="ExternalOutput")

    acts_rows = acts_d[:].rearrange("t b v -> (t b) v")     # (2048, 4096)
    acts_flat = bass.AP(tensor=acts_d[:].tensor, offset=0,
                        ap=[[1, T * BC * V], [1, 1]])

    with tile.TileContext(nc) as tc:
        with (
            tc.tile_pool(name="small", bufs=1) as small,
            tc.tile_pool(name="gidxp", bufs=2) as gidxp,
            tc.tile_pool(name="big", bufs=1) as big,
            tc.tile_pool(name="up", bufs=3) as up,
            tc.tile_pool(name="stream", bufs=2) as stream,
            tc.tile_pool(name="psum", bufs=1, space="PSUM") as psump,
        ):
            # ---------------- persistent tiles ----------------
            graw = big.tile([BC, J * T], F32)      # gathered acts
            E = big.tile([BC, J * T], BF16)        # tilted exp(gathered)
            ring1 = big.tile([BC, RING * (TH + 1)], F32)
            ring2 = big.tile([BC, RING * (TH + 1)], F32)

            skipm_t = small.tile([BC, S], F32)
            nc.sync.dma_start(out=skipm_t[:], in_=skipm_d[:])
            negc = small.tile([BC, 1], F32)
            nc.vector.memset(negc[:], -C_TILT)
            zbias = small.tile([128, 1], F32)
            nc.vector.memset(zbias[:], 0.0)
            zeros_t = small.tile([BC, T], F32)
            nc.vector.memset(zeros_t[:], 0.0)
            gm = small.tile([BC, T], F32)
            gm_a = small.tile([BC, T], F32)
            gm_b = small.tile([BC, T], F32)
            gmp = [small.tile([BC, T], F32, tag=f"gmp{i}") for i in range(CHUNKS)]
            gcum_t = small.tile([BC, T], F32)
            bound = small.tile([BC, S], F32)
            bsc = small.tile([BC, S], F32)
            m_t = small.tile([BC, 1], F32)
            r0_t = small.tile([BC, 1], F32)
            r_t = small.tile([BC, 1], F32)
            sums = small.tile([128, NT], F32)

            # ring init: zeros everywhere; virtual col s=-1 (ring pos 1)
            # slot0 = 1 seeds alpha0 through the uniform recurrence.
            nc.vector.memset(ring1[:], 0.0)
            nc.vector.memset(ring2[:], 0.0)
            nc.vector.memset(ring1[:, (TH + 1):(TH + 1) + 1], 1.0)

            # ---------------- gather + tilt + E ----------------
            for qi, (j0, nj) in enumerate(_j_chunks()):
                n_el = nj * T
                gq = gidxp.tile([BC, n_el], I32, tag="gidx")
                nc.sync.dma_start(out=gq[:], in_=gidx_d[:, j0 * T:(j0 + nj) * T])
                nc.gpsimd.indirect_dma_start(
                    out=graw[:, j0 * T:(j0 + nj) * T],
                    out_offset=None,
                    in_=acts_flat,
                    in_offset=bass.IndirectOffsetOnAxis(ap=gq[:], axis=0),
                )
                # chunk max over j (innermost after AP reorder)
                v3 = graw[:, j0 * T:(j0 + nj) * T].rearrange(
                    "p (j t) -> p t j", t=T)
                nc.vector.tensor_reduce(
                    out=gmp[qi][:], in_=v3, axis=mybir.AxisListType.X,
                    op=mybir.AluOpType.max)
            # combine chunk maxima (tree)
            nc.vector.tensor_tensor(out=gm_a[:], in0=gmp[0][:], in1=gmp[1][:],
                                    op=mybir.AluOpType.max)
            for i in range(2, CHUNKS):
                dst = gm_b if i % 2 == 0 else gm_a
                src = gm_a if i % 2 == 0 else gm_b
                nc.vector.tensor_tensor(out=dst[:], in0=src[:], in1=gmp[i][:],
                                        op=mybir.AluOpType.max)
            gm_fin = gm_b if CHUNKS % 2 == 0 else gm_a
            nc.vector.tensor_copy(out=gm[:], in_=gm_fin[:])

            # cumsum of gm for the host-side tilt correction
            nc.vector.tensor_tensor_scan(
                out=gcum_t[:], data0=gm[:], data1=zeros_t[:], initial=0.0,
                op0=mybir.AluOpType.add, op1=mybir.AluOpType.add)
            nc.sync.dma_start(out=gcum_d[:], in_=gcum_t[:])

            # E = exp(graw - gm - C_TILT)  (sub on GpSimd, exp in-place on ACT)
            for (j0, nj) in _j_chunks():
                sl = slice(j0 * T, (j0 + nj) * T)
                gm_bc = bass.AP(tensor=gm[:].tensor, offset=gm[:].offset,
                                ap=[gm[:].ap[0], [0, nj], gm[:].ap[1]])
                nc.gpsimd.tensor_tensor(
                    out=E[:, sl].rearrange("p (j t) -> p j t", t=T),
                    in0=graw[:, sl].rearrange("p (j t) -> p j t", t=T),
                    in1=gm_bc,
                    op=mybir.AluOpType.subtract)
                nc.scalar.activation(
                    out=E[:, sl], in_=E[:, sl],
                    func=mybir.ActivationFunctionType.Exp,
                    bias=negc[:], scale=1.0)

            # ---------------- s-major scans, two halves ----------------
            def col_base(s):
                return ((s + 2) % RING) * (TH + 1)

            for h in (0, 1):
                ring = ring1 if h == 0 else ring2
                toff = h * TH
                if h == 1:
                    # renorm boundary state to e^TB_LOG
                    nc.vector.reduce_max(out=m_t[:], in_=bound[:],
                                         axis=mybir.AxisListType.X)
                    nc.vector.reciprocal(out=r0_t[:], in_=m_t[:])
                    nc.vector.tensor_scalar_mul(r_t[:], r0_t[:],
                                                float(np.exp(TB_LOG)))
                    nc.sync.dma_start(out=rfac_d[:], in_=r_t[:])
                    nc.vector.tensor_scalar_mul(bsc[:], bound[:], r_t[:, 0:1])
                for s in range(S):
                    base = col_base(s)
                    pm1 = col_base(s - 1)
                    pm2 = col_base(s - 2)
                    j_slot = 0 if s % 2 == 0 else (s - 1) // 2 + 1
                    e_sl = E[:, j_slot * T + toff: j_slot * T + toff + TH]
                    if h == 1:
                        nc.sync.dma_start(out=ring[:, base:base + 1],
                                          in_=bsc[:, s:s + 1])
                    if s % 2 == 1:
                        u = up.tile([BC, TH], F32, tag="u")
                        nc.vector.scalar_tensor_tensor(
                            out=u[:],
                            in0=ring[:, pm2:pm2 + TH],
                            scalar=skipm_t[:, s:s + 1],
                            in1=ring[:, pm1:pm1 + TH],
                            op0=mybir.AluOpType.mult,
                            op1=mybir.AluOpType.add)
                        d0 = u[:]
                    else:
                        d0 = ring[:, pm1:pm1 + TH]
                    init = 0.0 if h == 0 else bsc[:, s:s + 1]
                    nc.vector.tensor_tensor_scan(
                        out=ring[:, base + 1:base + 1 + TH],
                        data0=d0, data1=e_sl, initial=init,
                        op0=mybir.AluOpType.add, op1=mybir.AluOpType.mult)
                    nc.sync.dma_start(out=xcols_d[s, :, toff:toff + TH],
                                      in_=ring[:, base + 1:base + 1 + TH])
                    if h == 0:
                        nc.sync.dma_start(out=bound[:, s:s + 1],
                                          in_=ring[:, base + TH:base + TH + 1])

            # ---------------- lse stream ----------------
            for i in range(NT):
                xt = stream.tile([128, V], F32, tag="xt")
                nc.sync.dma_start(out=xt[:],
                                  in_=acts_rows[i * 128:(i + 1) * 128, :])
                ex = psump.tile([128, V], F32, tag="ex")
                nc.scalar.activation(
                    out=ex[:], in_=xt[:],
                    func=mybir.ActivationFunctionType.Exp,
                    bias=zbias[:], scale=1.0,
                    accum_out=sums[:, i:i + 1])
            nc.sync.dma_start(out=sums_d[:], in_=sums[:])

    nc.compile()
    return nc


def _get_nc():
    if "nc" not in _CACHE:
        _CACHE["nc"] = _build_nc()
    return _CACHE["nc"]


def host_prep(acts, labels, act_lens, label_lens):
    """Build the 8 per-core input maps."""
    acts = np.ascontiguousarray(np.asarray(acts, dtype=np.float32))
    labels = np.asarray(labels).astype(np.int64)
    al = np.asarray(act_lens).astype(np.int64)
    ll = np.asarray(label_lens).astype(np.int64)
    offsets = np.cumsum(ll) - ll
    tgrid = np.arange(T, dtype=np.int64)
    in_maps = []
    for k in range(NCORES):
        bsl = slice(k * BC, (k + 1) * BC)
        slab = np.ascontiguousarray(acts[:, bsl, :])
        gidx = np.zeros((BC, J * T), np.int64)
        skipm = np.zeros((BC, S), np.float32)
        for bl in range(BC):
            b = k * BC + bl
            L = int(ll[b])
            lab = np.zeros(LMAX, np.int64)
            lab[:L] = labels[offsets[b]: offsets[b] + L]
            vs = np.concatenate([[0], lab])          # (J,)
            gidx[bl] = (tgrid[None, :] * (BC * V) + bl * V
                        + vs[:, None]).reshape(-1)
            skipm[bl, 1] = 1.0
            for jj in range(1, L):
                if lab[jj] != lab[jj - 1]:
                    skipm[bl, 2 * jj + 1] = 1.0
        in_maps.append({"acts": slab,
                        "gidx": gidx.astype(np.int32),
                        "skipm": skipm})
    return in_maps, al, ll


def host_finalize(results, al, ll):
    """Assemble the scalar loss from per-core outputs."""
    total = np.float64(0.0)
    for k in range(NCORES):
        r = results[k]
        sums = np.asarray(r["sums"], np.float64)          # (128, NT)
        xcols = np.asarray(r["xcols"], np.float64)        # (S, BC, T)
        gcum = np.asarray(r["gcum"], np.float64)          # (BC, T)
        rfac = np.asarray(r["rfac"], np.float64)          # (BC, 1)
        lse_rows = np.log(sums.T.reshape(-1)).reshape(T, BC)
        for bl in range(BC):
            b = k * BC + bl
            L = int(ll[b])
            albb = int(al[b])
            t_star = albb - 1
            e_s = 2 * L
            rs = xcols[e_s, bl, t_star] + xcols[e_s - 1, bl, t_star]
            log_unnorm = (np.log(rs) + gcum[bl, t_star]
                          + C_TILT * (t_star + 1))
            if t_star >= TH:
                log_unnorm -= np.log(rfac[bl, 0])
            loss_b = -log_unnorm + lse_rows[:albb, bl].sum()
            total += loss_b
    return np.array([total], dtype=np.float32)


def kernel(acts, labels, act_lens, label_lens):
    from concourse.bass_utils import run_bass_kernel_spmd
    in_maps, al, ll = host_prep(acts, labels, act_lens, label_lens)
    nc = _get_nc()
    res = run_bass_kernel_spmd(nc, in_maps, list(range(NCORES)))
    return host_finalize(res.results, al, ll)


# revision 1
# speedup vs baseline: 2.7115x; 2.7115x over previous
"""CTC total-loss kernel for Trainium2 (8 NeuronCores, Bass/Tile).

Strategy (data-parallel over batch, 4 examples per core):

 * The softmax denominator decouples from the CTC alpha recursion in the
   probability domain:  loss_b = -log(l1u + l2u) + sum_{t<al} lse[t,b] - tilt,
   where l*u come from an UNNORMALIZED recursion over exp(acts at lattice
   labels).  So each core runs two independent pipelines:
     1. stream its 33.5MB acts slab once, computing per-(t,b) sum(exp(acts))
        with a single fused ACT Exp+accum instruction per (128,4096) tile;
     2. gather the 33 lattice vocab columns per example (indirect DMA),
        and run the alpha recursion.
 * The alpha recursion is computed s-major: column s over all t is a
   first-order linear recurrence x_t = E_t * (x_{t-1} + u_t), one
   tensor_tensor_scan instruction per (column, half).  65 columns replace
   512 serial timesteps; shifts in s are free AP offsets.
 * f32 dynamic range is controlled by a per-(b,t) exponential tilt
   gamma_t = max_j(gathered acts)[t] + C_TILT, plus one renormalization of
   the boundary state at t=256 to a mid-window target.  Correction terms
   (cumsum of gamma, renorm factor) are outputs; the host folds them back
   in log domain (validated margins ~>8 nats to f32 limits on the
   reference input distribution).

The device program is input-independent (all data dependence flows through
input tensors), so it SPMDs across the 8 cores and compiles once.
Host work is index prep (labels -> gather indices / skip masks) and the
final ~100-flop log-domain assembly of the scalar loss.
"""

import numpy as np

import concourse.bass as bass
import concourse.bacc as bacc
import concourse.tile as tile
from concourse import mybir

F32 = mybir.dt.float32
BF16 = mybir.dt.bfloat16
I32 = mybir.dt.int32

T, B, V, LMAX = 512, 32, 4096, 32
NCORES = 8
BC = B // NCORES            # 4 examples per core
S = 2 * LMAX + 1            # 65 lattice states
J = LMAX + 1                # 33 gathered vocab slots (blank + labels)
TH = T // 2                 # 256: renorm halfway
NT = (T * BC) // 128        # 16 stream tiles of (128, V)
RING = 8                    # ring depth (columns kept in SBUF)
C_TILT = -1.20              # tilt constant on top of per-t max
TB_LOG = 58.0               # renorm boundary target: max -> e^TB_LOG
CHUNKS = 8                  # gather chunks (j-aligned)

_CACHE = {}


def _j_chunks():
    # split J=33 slots into CHUNKS j-aligned chunks
    base = J // CHUNKS
    sizes = [base] * CHUNKS
    for i in range(J - base * CHUNKS):
        sizes[i] += 1
    out = []
    j0 = 0
    for sz in sizes:
        out.append((j0, sz))
        j0 += sz
    return out


def _build_nc():
    nc = bacc.Bacc(None)
    acts_d = nc.dram_tensor("acts", [T, BC, V], F32, kind="ExternalInput")
    gidx_d = nc.dram_tensor("gidx", [BC, J * T], I32, kind="ExternalInput")
    skipm_d = nc.dram_tensor("skipm", [BC, S], F32, kind="ExternalInput")
    xcols_d = nc.dram_tensor("xcols", [S, BC, T], F32, kind="ExternalOutput")
    gcum_d = nc.dram_tensor("gcum", [BC, T], F32, kind="ExternalOutput")
    rfac_d = nc.dram_tensor("rfac", [BC, 1], F32, kind="ExternalOutput")
    sums_d = nc.dram_tensor("sums", [128, NT], F32, kind="ExternalOutput")

    acts_rows = acts_d[:].rearrange("t b v -> (t b) v")     # (2048, 4096)
    acts_flat = bass.AP(tensor=acts_d[:].tensor, offset=0,
                        ap=[[1, T * BC * V], [1, 1]])

    with tile.TileContext(nc) as tc:
        with (
            tc.tile_pool(name="small", bufs=1) as small,
            tc.tile_pool(name="gidxp", bufs=2) as gidxp,
            tc.tile_pool(name="big", bufs=1) as big,
            tc.tile_pool(name="up", bufs=3) as up,
            tc.tile_pool(name="stream", bufs=2) as stream,
            tc.tile_pool(name="psum", bufs=1, space="PSUM") as psump,
        ):
            # ---------------- persistent tiles ----------------
            graw = big.tile([BC, J * T], F32)      # gathered acts
            E = big.tile([BC, J * T], BF16)        # tilted exp(gathered)
            ring1 = big.tile([BC, RING * (TH + 1)], F32)
            ring2 = big.tile([BC, RING * (TH + 1)], F32)

            skipm_t = small.tile([BC, S], F32)
            nc.sync.dma_start(out=skipm_t[:], in_=skipm_d[:])
            negc = small.tile([BC, 1], F32)
            nc.vector.memset(negc[:], -C_TILT)
            zbias = small.tile([128, 1], F32)
            nc.vector.memset(zbias[:], 0.0)
            zeros_t = small.tile([BC, T], F32)
            nc.vector.memset(zeros_t[:], 0.0)
            gm = small.tile([BC, T], F32)
            gm_a = small.tile([BC, T], F32)
            gm_b = small.tile([BC, T], F32)
            gmp = [small.tile([BC, T], F32, tag=f"gmp{i}") for i in range(CHUNKS)]
            gcum_t = small.tile([BC, T], F32)
            bound = small.tile([BC, S], F32)
            bsc = small.tile([BC, S], F32)
            m_t = small.tile([BC, 1], F32)
            r0_t = small.tile([BC, 1], F32)
            r_t = small.tile([BC, 1], F32)
            sums = small.tile([128, NT], F32)

            # ring init: zeros everywhere; virtual col s=-1 (ring pos 1)
            # slot0 = 1 seeds alpha0 through the uniform recurrence.
            nc.vector.memset(ring1[:], 0.0)
            nc.vector.memset(ring2[:], 0.0)
            nc.vector.memset(ring1[:, (TH + 1):(TH + 1) + 1], 1.0)

            # ---------------- gather + tilt + E ----------------
            for qi, (j0, nj) in enumerate(_j_chunks()):
                n_el = nj * T
                gq = gidxp.tile([BC, n_el], I32, tag="gidx")
                nc.sync.dma_start(out=gq[:], in_=gidx_d[:, j0 * T:(j0 + nj) * T])
                nc.gpsimd.indirect_dma_start(
                    out=graw[:, j0 * T:(j0 + nj) * T],
                    out_offset=None,
                    in_=acts_flat,
                    in_offset=bass.IndirectOffsetOnAxis(ap=gq[:], axis=0),
                )
                # chunk max over j (innermost after AP reorder)
                v3 = graw[:, j0 * T:(j0 + nj) * T].rearrange(
                    "p (j t) -> p t j", t=T)
                nc.vector.tensor_reduce(
                    out=gmp[qi][:], in_=v3, axis=mybir.AxisListType.X,
                    op=mybir.AluOpType.max)
            # combine chunk maxima (tree)
            nc.vector.tensor_tensor(out=gm_a[:], in0=gmp[0][:], in1=gmp[1][:],
                                    op=mybir.AluOpType.max)
            for i in range(2, CHUNKS):
                dst = gm_b if i % 2 == 0 else gm_a
                src = gm_a if i % 2 == 0 else gm_b
                nc.vector.tensor_tensor(out=dst[:], in0=src[:], in1=gmp[i][:],
                                        op=mybir.AluOpType.max)
            gm_fin = gm_b if CHUNKS % 2 == 0 else gm_a
            nc.vector.tensor_copy(out=gm[:], in_=gm_fin[:])

            # cumsum of gm for the host-side tilt correction
            nc.vector.tensor_tensor_scan(
                out=gcum_t[:], data0=gm[:], data1=zeros_t[:], initial=0.0,
                op0=mybir.AluOpType.add, op1=mybir.AluOpType.add)
            nc.sync.dma_start(out=gcum_d[:], in_=gcum_t[:])

            # E = exp(graw - gm - C_TILT)  (sub on GpSimd, exp in-place on ACT)
            for (j0, nj) in _j_chunks():
                sl = slice(j0 * T, (j0 + nj) * T)
                gm_bc = bass.AP(tensor=gm[:].tensor, offset=gm[:].offset,
                                ap=[gm[:].ap[0], [0, nj], gm[:].ap[1]])
                nc.gpsimd.tensor_tensor(
                    out=E[:, sl].rearrange("p (j t) -> p j t", t=T),
                    in0=graw[:, sl].rearrange("p (j t) -> p j t", t=T),
                    in1=gm_bc,
                    op=mybir.AluOpType.subtract)
                nc.scalar.activation(
                    out=E[:, sl], in_=E[:, sl],
                    func=mybir.ActivationFunctionType.Exp,
                    bias=negc[:], scale=1.0)

            # ---------------- s-major scans, two halves ----------------
            def col_base(s):
                return ((s + 2) % RING) * (TH + 1)

            for h in (0, 1):
                ring = ring1 if h == 0 else ring2
                toff = h * TH
                if h == 1:
                    # renorm boundary state to e^TB_LOG
                    nc.vector.reduce_max(out=m_t[:], in_=bound[:],
                                         axis=mybir.AxisListType.X)
                    nc.vector.reciprocal(out=r0_t[:], in_=m_t[:])
                    nc.vector.tensor_scalar_mul(r_t[:], r0_t[:],
                                                float(np.exp(TB_LOG)))
                    nc.sync.dma_start(out=rfac_d[:], in_=r_t[:])
                    nc.vector.tensor_scalar_mul(bsc[:], bound[:], r_t[:, 0:1])
                for s in range(S):
                    base = col_base(s)
                    pm1 = col_base(s - 1)
                    pm2 = col_base(s - 2)
                    j_slot = 0 if s % 2 == 0 else (s - 1) // 2 + 1
                    e_sl = E[:, j_slot * T + toff: j_slot * T + toff + TH]
                    if h == 1:
                        nc.sync.dma_start(out=ring[:, base:base + 1],
                                          in_=bsc[:, s:s + 1])
                    if s % 2 == 1:
                        u = up.tile([BC, TH], F32, tag="u")
                        nc.vector.scalar_tensor_tensor(
                            out=u[:],
                            in0=ring[:, pm2:pm2 + TH],
                            scalar=skipm_t[:, s:s + 1],
                            in1=ring[:, pm1:pm1 + TH],
                            op0=mybir.AluOpType.mult,
                            op1=mybir.AluOpType.add)
                        d0 = u[:]
                    else:
                        d0 = ring[:, pm1:pm1 + TH]
                    init = 0.0 if h == 0 else bsc[:, s:s + 1]
                    nc.vector.tensor_tensor_scan(
                        out=ring[:, base + 1:base + 1 + TH],
                        data0=d0, data1=e_sl, initial=init,
                        op0=mybir.AluOpType.add, op1=mybir.AluOpType.mult)
                    nc.sync.dma_start(out=xcols_d[s, :, toff:toff + TH],
                                      in_=ring[:, base + 1:base + 1 + TH])
                    if h == 0:
                        nc.sync.dma_start(out=bound[:, s:s + 1],
                                          in_=ring[:, base + TH:base + TH + 1])

            # ---------------- lse stream ----------------
            for i in range(NT):
                xt = stream.tile([128, V], F32, tag="xt")
                nc.sync.dma_start(out=xt[:],
                                  in_=acts_rows[i * 128:(i + 1) * 128, :])
                ex = psump.tile([128, V], F32, tag="ex")
                nc.scalar.activation(
                    out=ex[:], in_=xt[:],
                    func=mybir.ActivationFunctionType.Exp,
                    bias=zbias[:], scale=1.0,
                    accum_out=sums[:, i:i + 1])
            nc.sync.dma_start(out=sums_d[:], in_=sums[:])

    nc.compile()
    return nc


def _get_nc():
    if "nc" not in _CACHE:
        _CACHE["nc"] = _build_nc()
    return _CACHE["nc"]


def host_prep(acts, labels, act_lens, label_lens):
    """Build the 8 per-core input maps."""
    acts = np.ascontiguousarray(np.asarray(acts, dtype=np.float32))
    labels = np.asarray(labels).astype(np.int64)
    al = np.asarray(act_lens).astype(np.int64)
    ll = np.asarray(label_lens).astype(np.int64)
    offsets = np.cumsum(ll) - ll
    tgrid = np.arange(T, dtype=np.int64)
    in_maps = []
    for k in range(NCORES):
        bsl = slice(k * BC, (k + 1) * BC)
        slab = np.ascontiguousarray(acts[:, bsl, :])
        gidx = np.zeros((BC, J * T), np.int64)
        skipm = np.zeros((BC, S), np.float32)
        for bl in range(BC):
            b = k * BC + bl
            L = int(ll[b])
            lab = np.zeros(LMAX, np.int64)
            lab[:L] = labels[offsets[b]: offsets[b] + L]
            vs = np.concatenate([[0], lab])          # (J,)
            gidx[bl] = (tgrid[None, :] * (BC * V) + bl * V
                        + vs[:, None]).reshape(-1)
            skipm[bl, 1] = 1.0
            for jj in range(1, L):
                if lab[jj] != lab[jj - 1]:
                    skipm[bl, 2 * jj + 1] = 1.0
        in_maps.append({"acts": slab,
                        "gidx": gidx.astype(np.int32),
                        "skipm": skipm})
    return in_maps, al, ll


def host_finalize(results, al, ll):
    """Assemble the scalar loss from per-core outputs."""
    total = np.float64(0.0)
    for k in range(NCORES):
        r = results[k]
        sums = np.asarray(r["sums"], np.float64)          # (128, NT)
        xcols = np.asarray(r["xcols"], np.float64)        # (S, BC, T)
        gcum = np.asarray(r["gcum"], np.float64)          # (BC, T)
        rfac = np.asarray(r["rfac"], np.float64)          # (BC, 1)
        lse_rows = np.log(sums.T.reshape(-1)).reshape(T, BC)
        for bl in range(BC):
            b = k * BC + bl
            L = int(ll[b])
            albb = int(al[b])
            t_star = albb - 1
            e_s = 2 * L
            rs = xcols[e_s, bl, t_star] + xcols[e_s - 1, bl, t_star]
            log_unnorm = (np.log(rs) + gcum[bl, t_star]
                          + C_TILT * (t_star + 1))
            if t_star >= TH:
                log_unnorm -= np.log(rfac[bl, 0])
            loss_b = -log_unnorm + lse_rows[:albb, bl].sum()
            total += loss_b
    return np.array([total], dtype=np.float32)


def kernel(acts, labels, act_lens, label_lens):
    from concourse.bass_utils import run_bass_kernel_spmd
    in_maps, al, ll = host_prep(acts, labels, act_lens, label_lens)
    nc = _get_nc()
    res = run_bass_kernel_spmd(nc, in_maps, list(range(NCORES)))
    return host_finalize(res.results, al, ll)


# revision 6
# speedup vs baseline: 7.6774x; 2.8314x over previous
"""CTC total-loss kernel for Trainium2 (8 NeuronCores, Bass/Tile).

Strategy (data-parallel over batch, 4 examples per core):

 * The softmax denominator decouples from the CTC alpha recursion in the
   probability domain:  loss_b = -log(l1u + l2u) + sum_{t<al} lse[t,b] - tilt,
   where l*u come from an UNNORMALIZED recursion over exp(acts at lattice
   labels).  So each core runs two independent pipelines:
     1. stream its 33.5MB acts slab once, computing per-(t,b) sum(exp(acts))
        with a single fused ACT Exp+accum instruction per (128,4096) tile;
     2. gather the 33 lattice vocab columns per example (indirect DMA),
        and run the alpha recursion.
 * The alpha recursion is computed s-major: column s over all t is a
   first-order linear recurrence x_t = E_t * (x_{t-1} + u_t), one
   tensor_tensor_scan instruction per (column, half).  65 columns replace
   512 serial timesteps; shifts in s are free AP offsets.
 * f32 dynamic range is controlled by a per-(b,t) exponential tilt
   gamma_t = max_j(gathered acts)[t] + C_TILT, plus one renormalization of
   the boundary state at t=256 to a mid-window target.  Correction terms
   (cumsum of gamma, renorm factor) are outputs; the host folds them back
   in log domain (validated margins ~>8 nats to f32 limits on the
   reference input distribution).

The device program is input-independent (all data dependence flows through
input tensors), so it SPMDs across the 8 cores and compiles once.
Host work is index prep (labels -> gather indices / skip masks) and the
final ~100-flop log-domain assembly of the scalar loss.
"""

import numpy as np

import concourse.bass as bass
import concourse.bacc as bacc
import concourse.tile as tile
from concourse import mybir

F32 = mybir.dt.float32
BF16 = mybir.dt.bfloat16
I32 = mybir.dt.int32

T, B, V, LMAX = 512, 32, 4096, 32
NCORES = 8
BC = B // NCORES            # 4 examples per core
S = 2 * LMAX + 1            # 65 lattice states
J = LMAX + 1                # 33 gathered vocab slots (blank + labels)
TH = T // 2                 # 256: renorm halfway
NT = (T * BC) // 128        # 16 stream tiles of (128, V)
RING = 8                    # ring depth (columns kept in SBUF)
C_TILT = -1.20              # tilt constant on top of per-t max
TB_LOG = 58.0               # renorm boundary target: max -> e^TB_LOG
CHUNKS = 4                  # j-chunks for the E pipeline

_CACHE = {}


def _j_chunks():
    # split J=33 slots into CHUNKS j-aligned chunks
    base = J // CHUNKS
    sizes = [base] * CHUNKS
    for i in range(J - base * CHUNKS):
        sizes[i] += 1
    out = []
    j0 = 0
    for sz in sizes:
        out.append((j0, sz))
        j0 += sz
    return out


def _build_nc():
    nc = bacc.Bacc(None)
    acts_d = nc.dram_tensor("acts", [T, BC, V], F32, kind="ExternalInput")
    ridx_d = nc.dram_tensor("ridx", [J, BC], I32, kind="ExternalInput")
    skipm_d = nc.dram_tensor("skipm", [BC, S], F32, kind="ExternalInput")
    xcols_d = nc.dram_tensor("xcols", [S, BC, T], F32, kind="ExternalOutput")
    gcum_d = nc.dram_tensor("gcum", [BC, T], F32, kind="ExternalOutput")
    rfac_d = nc.dram_tensor("rfac", [BC, 1], F32, kind="ExternalOutput")
    sums_d = nc.dram_tensor("sums", [128, NT], F32, kind="ExternalOutput")

    acts_rows = acts_d[:].rearrange("t b v -> (t b) v")     # (2048, 4096)
    # transposed row view for the gather: row r = b*V + v (stride 1),
    # inner dim walks t at stride BC*V; trailing [1,1] keeps the DMA
    # layer's last-dim-contiguity check happy.
    acts_tv = bass.AP(tensor=acts_d[:].tensor, offset=0,
                      ap=[[1, BC * V], [BC * V, T], [1, 1]])

    with tile.TileContext(nc) as tc:
        with (
            tc.tile_pool(name="small", bufs=1) as small,
            tc.tile_pool(name="gbp", bufs=2) as gbp,
            tc.tile_pool(name="big", bufs=1) as big,
            tc.tile_pool(name="up", bufs=3) as up,
            tc.tile_pool(name="stream", bufs=3) as stream,
            tc.tile_pool(name="psum", bufs=1, space="PSUM") as psump,
        ):
            # ---------------- persistent tiles ----------------
            graw = big.tile([BC, J * T], F32)      # gathered acts
            E = big.tile([BC, J * T], BF16)        # tilted exp(gathered)
            ring1 = big.tile([BC, RING * (TH + 1)], F32)
            ring2 = big.tile([BC, RING * (TH + 1)], F32)

            skipm_t = small.tile([BC, S], F32)
            nc.sync.dma_start(out=skipm_t[:], in_=skipm_d[:])
            negc = small.tile([BC, 1], F32)
            nc.vector.memset(negc[:], -C_TILT)
            zbias = small.tile([128, 1], F32)
            nc.vector.memset(zbias[:], 0.0)
            zeros_t = small.tile([BC, T], F32)
            nc.vector.memset(zeros_t[:], 0.0)
            gm = small.tile([BC, T], F32)
            gcum_t = small.tile([BC, T], F32)
            bound = small.tile([BC, S], F32)
            bsc = small.tile([BC, S], F32)
            m_t = small.tile([BC, 1], F32)
            r0_t = small.tile([BC, 1], F32)
            r_t = small.tile([BC, 1], F32)
            sums = small.tile([128, NT], F32)

            # ring init: zeros everywhere; virtual col s=-1 (ring pos 1)
            # slot0 = 1 seeds alpha0 through the uniform recurrence.
            nc.vector.memset(ring1[:], 0.0)
            nc.vector.memset(ring2[:], 0.0)
            nc.vector.memset(ring1[:, (TH + 1):(TH + 1) + 1], 1.0)

            # ---------------- gather + tilt + E ----------------
            ridx_t = small.tile([J, BC], I32)
            nc.sync.dma_start(out=ridx_t[:], in_=ridx_d[:])
            for b in range(BC):
                gb = gbp.tile([J, T], F32, tag="gb")
                nc.gpsimd.indirect_dma_start(
                    out=gb[:],
                    out_offset=None,
                    in_=acts_tv,
                    in_offset=bass.IndirectOffsetOnAxis(
                        ap=ridx_t[:, b:b + 1], axis=0),
                )
                # flatten (J, T) -> row b of graw (cross-partition via DMA)
                nc.sync.dma_start(out=graw[b:b + 1, :], in_=gb[:, :])

            # gm[t] = max_j graw[b, j, t]
            v3 = graw[:].rearrange("p (j t) -> p t j", t=T)
            nc.vector.tensor_reduce(
                out=gm[:], in_=v3, axis=mybir.AxisListType.X,
                op=mybir.AluOpType.max)

            # cumsum of gm for the host-side tilt correction
            nc.vector.tensor_tensor_scan(
                out=gcum_t[:], data0=gm[:], data1=zeros_t[:], initial=0.0,
                op0=mybir.AluOpType.add, op1=mybir.AluOpType.add)
            nc.sync.dma_start(out=gcum_d[:], in_=gcum_t[:])

            # E = exp(graw - gm - C_TILT)  (sub on GpSimd, exp in-place on ACT)
            for (j0, nj) in _j_chunks():
                sl = slice(j0 * T, (j0 + nj) * T)
                gm_bc = bass.AP(tensor=gm[:].tensor, offset=gm[:].offset,
                                ap=[gm[:].ap[0], [0, nj], gm[:].ap[1]])
                nc.gpsimd.tensor_tensor(
                    out=E[:, sl].rearrange("p (j t) -> p j t", t=T),
                    in0=graw[:, sl].rearrange("p (j t) -> p j t", t=T),
                    in1=gm_bc,
                    op=mybir.AluOpType.subtract)
                nc.scalar.activation(
                    out=E[:, sl], in_=E[:, sl],
                    func=mybir.ActivationFunctionType.Exp,
                    bias=negc[:], scale=1.0)

            # ---------------- s-major scans, two halves ----------------
            def col_base(s):
                return ((s + 2) % RING) * (TH + 1)

            for h in (0, 1):
                ring = ring1 if h == 0 else ring2
                toff = h * TH
                if h == 1:
                    # renorm boundary state to e^TB_LOG
                    nc.vector.reduce_max(out=m_t[:], in_=bound[:],
                                         axis=mybir.AxisListType.X)
                    nc.vector.reciprocal(out=r0_t[:], in_=m_t[:])
                    nc.vector.tensor_scalar_mul(r_t[:], r0_t[:],
                                                float(np.exp(TB_LOG)))
                    nc.sync.dma_start(out=rfac_d[:], in_=r_t[:])
                    nc.vector.tensor_scalar_mul(bsc[:], bound[:], r_t[:, 0:1])
                for s in range(S):
                    base = col_base(s)
                    pm1 = col_base(s - 1)
                    pm2 = col_base(s - 2)
                    j_slot = 0 if s % 2 == 0 else (s - 1) // 2 + 1
                    e_sl = E[:, j_slot * T + toff: j_slot * T + toff + TH]
                    if h == 1:
                        nc.sync.dma_start(out=ring[:, base:base + 1],
                                          in_=bsc[:, s:s + 1])
                    if s % 2 == 1:
                        u = up.tile([BC, TH], F32, tag="u")
                        nc.vector.scalar_tensor_tensor(
                            out=u[:],
                            in0=ring[:, pm2:pm2 + TH],
                            scalar=skipm_t[:, s:s + 1],
                            in1=ring[:, pm1:pm1 + TH],
                            op0=mybir.AluOpType.mult,
                            op1=mybir.AluOpType.add)
                        d0 = u[:]
                    else:
                        d0 = ring[:, pm1:pm1 + TH]
                    init = 0.0 if h == 0 else bsc[:, s:s + 1]
                    nc.vector.tensor_tensor_scan(
                        out=ring[:, base + 1:base + 1 + TH],
                        data0=d0, data1=e_sl, initial=init,
                        op0=mybir.AluOpType.add, op1=mybir.AluOpType.mult)
                    nc.sync.dma_start(out=xcols_d[s, :, toff:toff + TH],
                                      in_=ring[:, base + 1:base + 1 + TH])
                    if h == 0:
                        nc.sync.dma_start(out=bound[:, s:s + 1],
                                          in_=ring[:, base + TH:base + TH + 1])

            # ---------------- lse stream ----------------
            for i in range(NT):
                xt = stream.tile([128, V], F32, tag="xt")
                nc.sync.dma_start(out=xt[:],
                                  in_=acts_rows[i * 128:(i + 1) * 128, :])
                ex = psump.tile([128, V], F32, tag="ex")
                nc.scalar.activation(
                    out=ex[:], in_=xt[:],
                    func=mybir.ActivationFunctionType.Exp,
                    bias=zbias[:], scale=1.0,
                    accum_out=sums[:, i:i + 1])
            nc.sync.dma_start(out=sums_d[:], in_=sums[:])

    nc.compile()
    return nc


def _get_nc():
    if "nc" not in _CACHE:
        _CACHE["nc"] = _build_nc()
    return _CACHE["nc"]


def host_prep(acts, labels, act_lens, label_lens):
    """Build the 8 per-core input maps."""
    acts = np.ascontiguousarray(np.asarray(acts, dtype=np.float32))
    labels = np.asarray(labels).astype(np.int64)
    al = np.asarray(act_lens).astype(np.int64)
    ll = np.asarray(label_lens).astype(np.int64)
    offsets = np.cumsum(ll) - ll
    in_maps = []
    for k in range(NCORES):
        bsl = slice(k * BC, (k + 1) * BC)
        slab = np.ascontiguousarray(acts[:, bsl, :])
        ridx = np.zeros((J, BC), np.int32)
        skipm = np.zeros((BC, S), np.float32)
        for bl in range(BC):
            b = k * BC + bl
            L = int(ll[b])
            lab = np.zeros(LMAX, np.int64)
            lab[:L] = labels[offsets[b]: offsets[b] + L]
            vs = np.concatenate([[0], lab])          # (J,)
            ridx[:, bl] = bl * V + vs
            skipm[bl, 1] = 1.0
            for jj in range(1, L):
                if lab[jj] != lab[jj - 1]:
                    skipm[bl, 2 * jj + 1] = 1.0
        in_maps.append({"acts": slab, "ridx": ridx, "skipm": skipm})
    return in_maps, al, ll


def host_finalize(results, al, ll):
    """Assemble the scalar loss from per-core outputs."""
    total = np.float64(0.0)
    for k in range(NCORES):
        r = results[k]
        sums = np.asarray(r["sums"], np.float64)          # (128, NT)
        xcols = np.asarray(r["xcols"], np.float64)        # (S, BC, T)
        gcum = np.asarray(r["gcum"], np.float64)          # (BC, T)
        rfac = np.asarray(r["rfac"], np.float64)          # (BC, 1)
        lse_rows = np.log(sums.T.reshape(-1)).reshape(T, BC)
        for bl in range(BC):
            b = k * BC + bl
            L = int(ll[b])
            albb = int(al[b])
            t_star = albb - 1
            e_s = 2 * L
            rs = xcols[e_s, bl, t_star] + xcols[e_s - 1, bl, t_star]
            log_unnorm = (np.log(rs) + gcum[bl, t_star]
                          + C_TILT * (t_star + 1))
            if t_star >= TH:
                log_unnorm -= np.log(rfac[bl, 0])
            loss_b = -log_unnorm + lse_rows[:albb, bl].sum()
            total += loss_b
    return np.array([total], dtype=np.float32)


def kernel(acts, labels, act_lens, label_lens):
    from concourse.bass_utils import run_bass_kernel_spmd
    in_maps, al, ll = host_prep(acts, labels, act_lens, label_lens)
    nc = _get_nc()
    res = run_bass_kernel_spmd(nc, in_maps, list(range(NCORES)))
    return host_finalize(res.results, al, ll)
